# revision 2
# baseline (speedup 1.0000x reference)
"""Trainium2 Bass kernel for nn_Conv2D3_72026601554290.

Reference computation:
    h = conv7x7_valid(x[4,3,70,70], W1[64,3,7,7]) + b1      -> [4,64,64,64]
    repeat 200x: h = W2 @ h + b2   (1x1 conv, shared weights)

Strategy:
  * The 200 repeated affine steps share one weight matrix, so the tail of
    the network is the affine map h -> W2^200 h + (sum_k W2^k) b2.  We fold
    W2^FUSE (computed in float64 on the host, rounded to f32) into the
    device program: the device runs 200/FUSE GEMM steps.  FUSE=200 folds
    everything into the conv weights (single fused conv).  Numerics vs the
    f32 reference are ~1e-6 for every FUSE (verified: spectral radius of
    W2 is 0.979, all paths stay tiny).
  * Data parallel across 8 NeuronCores: 16384 output positions -> 2048 per
    core (half an image each).  No cross-core communication.
  * Conv is an im2col GEMM: K=147 split into 128+19 accumulated matmuls.
    Positions are packed two-deep on the partition axis ([128, 1024]
    layout, position groups A/B), and A/B matmuls run concurrently on
    distinct PE-array column halves (tile_position packing).
  * Each loop step runs 4 concurrent 64x64 matmuls, one per PE-array
    quadrant; PSUM->SBUF bias-add copies are split between the Vector and
    Scalar engines.  The quadrant packing swaps two quarter blocks each
    step (an involution); host reassembly undoes it for odd step counts.
"""

import numpy as np

import concourse.bacc as bacc
import concourse.tile as tile
import concourse.mybir as mybir
from concourse.bass_utils import run_bass_kernel_spmd

F32 = mybir.dt.float32

N_CORES = 8
N_REPEAT = 200
FUSE = 200  # device runs N_REPEAT//FUSE GEMM steps; 200 -> fully folded conv
POS_PER_CORE = 2048  # 4*64*64 / 8
HALF = POS_PER_CORE // 2  # 1024, free-dim size of the [128, 1024] layout
OH = OW = 64
KH = KW = 7
CIN = 3
CH = 64
K_IM = CIN * KH * KW  # 147

_cache = {}


def _build_nc(steps):
    """Build + compile the per-core Bass program (same NEFF for all cores)."""
    nc = bacc.Bacc("TRN2", target_bir_lowering=False, debug=False,
                   num_devices=N_CORES)

    wc_hi_ext = nc.declare_dram_parameter("wc_hi", [128, CH], F32, isOutput=False)
    wc_lo_ext = nc.declare_dram_parameter("wc_lo", [K_IM - 128, CH], F32, isOutput=False)
    bc_ext = nc.declare_dram_parameter("bc", [128, 1], F32, isOutput=False)
    im_hi_ext = nc.declare_dram_parameter("im_hi", [128, POS_PER_CORE], F32, isOutput=False)
    im_lo_ext = nc.declare_dram_parameter("im_lo", [K_IM - 128, POS_PER_CORE], F32, isOutput=False)
    if steps:
        wl_ext = nc.declare_dram_parameter("wl", [128, CH], F32, isOutput=False)
        bl_ext = nc.declare_dram_parameter("bl", [128, 1], F32, isOutput=False)
    o_ext = nc.declare_dram_parameter("o", [128, HALF], F32, isOutput=True)

    with tile.TileContext(nc) as tc:
        with (
            tc.tile_pool(name="const", bufs=1) as cpool,
            tc.tile_pool(name="act", bufs=3) as apool,
            tc.tile_pool(name="psum", bufs=4, space="PSUM") as ppool,
        ):
            wc_hi = cpool.tile([128, CH], F32)
            wc_lo = cpool.tile([K_IM - 128, CH], F32)
            bc = cpool.tile([128, 1], F32)
            im_hi = cpool.tile([128, POS_PER_CORE], F32)
            im_lo = cpool.tile([K_IM - 128, POS_PER_CORE], F32)
            nc.sync.dma_start(wc_hi[:], wc_hi_ext[:])
            nc.sync.dma_start(wc_lo[:], wc_lo_ext[:])
            nc.sync.dma_start(bc[:], bc_ext[:])
            nc.sync.dma_start(im_hi[:], im_hi_ext[:])
            nc.sync.dma_start(im_lo[:], im_lo_ext[:])
            if steps:
                wl = cpool.tile([128, CH], F32)
                bl = cpool.tile([128, 1], F32)
                nc.sync.dma_start(wl[:], wl_ext[:])
                nc.sync.dma_start(bl[:], bl_ext[:])
            tc.strict_bb_all_engine_barrier()

            # ---- conv: out[ch, pos] = Wc^T @ im2col, packed A/B on col halves
            h = apool.tile([128, HALF], F32, tag="h")
            for half in range(2):  # free-dim halves of the [128, 1024] layout
                ps = ppool.tile([128, 512], F32, tag="ps")
                for grp in range(2):  # A -> psum partitions 0:64, B -> 64:128
                    off = grp * HALF + half * 512
                    ps_out = ps[grp * 64:(grp + 1) * 64, :]
                    nc.tensor.matmul(ps_out, wc_hi[:], im_hi[:, off:off + 512],
                                     start=True, stop=False,
                                     tile_position=(0, grp * 64))
                    nc.tensor.matmul(ps_out, wc_lo[:], im_lo[:, off:off + 512],
                                     start=False, stop=True,
                                     tile_position=(0, grp * 64))
                dst = h[:, half * 512:(half + 1) * 512]
                if half == 0:
                    nc.vector.tensor_scalar(dst, ps[:], bc[:], None,
                                            mybir.AluOpType.add)
                else:
                    nc.scalar.activation(dst, ps[:],
                                         mybir.ActivationFunctionType.Identity,
                                         bias=bc[:])

            # ---- fused GEMM steps: h <- P_FUSE @ h + c_FUSE
            for _ in range(steps):
                psA = ppool.tile([128, 512], F32, tag="ps")
                psB = ppool.tile([128, 512], F32, tag="ps")
                nc.tensor.matmul(psA[0:64, :], wl[0:64, :], h[0:64, 0:512],
                                 start=True, stop=True, tile_position=(0, 0))
                nc.tensor.matmul(psA[64:128, :], wl[0:64, :], h[0:64, 512:1024],
                                 start=True, stop=True, tile_position=(0, 64))
                nc.tensor.matmul(psB[0:64, :], wl[64:128, :], h[64:128, 0:512],
                                 start=True, stop=True, tile_position=(64, 0))
                nc.tensor.matmul(psB[64:128, :], wl[64:128, :], h[64:128, 512:1024],
                                 start=True, stop=True, tile_position=(64, 64))
                h_new = apool.tile([128, HALF], F32, tag="h")
                nc.vector.tensor_scalar(h_new[:, 0:512], psA[:], bl[:], None,
                                        mybir.AluOpType.add)
                nc.scalar.activation(h_new[:, 512:1024], psB[:],
                                     mybir.ActivationFunctionType.Identity,
                                     bias=bl[:])
                h = h_new

            nc.sync.dma_start(o_ext[:], h[:])

    nc.compile()
    return nc


def _fold(W1, b1, W2, b2, fuse):
    """Fold `fuse` affine steps into the conv weights (float64 host math).

    Returns (Wc [64,147], bc [64], Pk [64,64] or None, ck [64] or None):
      conv weights with FUSE steps pre-applied when fuse==N_REPEAT, else
      original conv weights plus the per-step fused matrix P_FUSE / c_FUSE.
    """
    W2d = W2.astype(np.float64)
    W1m = W1.reshape(CH, K_IM).astype(np.float64)

    def affine_pow(k):
        # (P, S) with P = W2^k, S = sum_{j<k} W2^j  via binary doubling
        P = np.eye(CH)
        S = np.zeros((CH, CH))
        base_P = W2d
        base_S = np.eye(CH)
        while k:
            if k & 1:
                S = base_S + base_P @ S
                P = base_P @ P
            base_S = base_S + base_P @ base_S
            base_P = base_P @ base_P
            k >>= 1
        return P, S

    if fuse == N_REPEAT:
        P, S = affine_pow(N_REPEAT)
        Wc = (P @ W1m).astype(np.float32)
        bc = (P @ b1.astype(np.float64) + S @ b2.astype(np.float64)).astype(np.float32)
        return Wc, bc, None, None
    P, S = affine_pow(fuse)
    return (W1m.astype(np.float32), b1.astype(np.float32),
            P.astype(np.float32), (S @ b2.astype(np.float64)).astype(np.float32))


def _im2col_core(x, core):
    """im2col for core's 2048 output positions -> [147, 2048] f32."""
    b = core // 2
    y0 = 32 * (core % 2)
    cols = np.empty((K_IM, POS_PER_CORE), np.float32)
    i = 0
    for c in range(CIN):
        for dy in range(KH):
            for dx in range(KW):
                # positions: y in [y0, y0+32), xo in [0, 64)
                cols[i] = x[b, c, y0 + dy:y0 + dy + 32, dx:dx + OW].reshape(-1)
                i += 1
    return cols


def _run(x, W1, b1, W2, b2, trace=False):
    x = np.asarray(x, dtype=np.float32)
    W1 = np.asarray(W1, dtype=np.float32)
    b1 = np.asarray(b1, dtype=np.float32)
    W2 = np.asarray(W2, dtype=np.float32)
    b2 = np.asarray(b2, dtype=np.float32)

    steps = N_REPEAT // FUSE if FUSE != N_REPEAT else 0
    assert FUSE * (steps or 1) == N_REPEAT or FUSE == N_REPEAT

    if "nc" not in _cache or _cache.get("steps") != steps:
        _cache["nc"] = _build_nc(steps)
        _cache["steps"] = steps
    nc = _cache["nc"]

    Wc, bc, Pk, ck = _fold(W1, b1, W2, b2, FUSE)
    WcT = np.ascontiguousarray(Wc.T)  # [147, 64] lhsT layout
    wc_hi = WcT[:128]
    wc_lo = WcT[128:]
    bc_dup = np.concatenate([bc, bc])[:, None].astype(np.float32)

    in_maps = []
    for core in range(N_CORES):
        cols = _im2col_core(x, core)
        m = {
            "wc_hi": wc_hi, "wc_lo": wc_lo, "bc": bc_dup,
            "im_hi": np.ascontiguousarray(cols[:128]),
            "im_lo": np.ascontiguousarray(cols[128:]),
        }
        if steps:
            PkT = np.ascontiguousarray(Pk.T)
            m["wl"] = np.concatenate([PkT, PkT], axis=0).astype(np.float32)
            m["bl"] = np.concatenate([ck, ck])[:, None].astype(np.float32)
        in_maps.append(m)

    res = run_bass_kernel_spmd(nc, in_maps, list(range(N_CORES)), trace=trace)

    out = np.empty((4, CH, OH, OW), np.float32)
    for core in range(N_CORES):
        o = res.results[core]["o"].copy()
        if steps % 2 == 1:
            tmp = o[0:64, 512:1024].copy()
            o[0:64, 512:1024] = o[64:128, 0:512]
            o[64:128, 0:512] = tmp
        b = core // 2
        y0 = 32 * (core % 2)
        # group A = local positions 0..1023 (16 rows), group B = 1024..2047
        out[b, :, y0:y0 + 16, :] = o[0:64].reshape(CH, 16, OW)
        out[b, :, y0 + 16:y0 + 32, :] = o[64:128].reshape(CH, 16, OW)
    return out, res


def kernel(**inputs):
    out, _ = _run(inputs["x"], inputs["W1"], inputs["b1"],
                  inputs["W2"], inputs["b2"], trace=False)
    return out


def kernel_traced(**inputs):
    """Like kernel() but with NTFF hardware profiling; returns (out, res)."""
    import sys
    import types
    if "antenv.axon_hooks" not in sys.modules:
        from trn_agent_boot.trn_boot import _ntff_profile_via_ctypes
        hook = _ntff_profile_via_ctypes("/opt/axon/libaxon_pjrt.so")
        mod = types.ModuleType("antenv.axon_hooks")
        mod.get_axon_ntff_profile_hook = lambda: hook
        mod.set_axon_ntff_profile_hook = lambda h: None
        sys.modules["antenv.axon_hooks"] = mod
    return _run(inputs["x"], inputs["W1"], inputs["b1"],
                inputs["W2"], inputs["b2"], trace=True)


# revision 9
# speedup vs baseline: 1.0907x; 1.0907x over previous
"""Trainium2 Bass kernel for nn_Conv2D3_72026601554290.

Reference computation:
    h = conv7x7_valid(x[4,3,70,70], W1[64,3,7,7]) + b1      -> [4,64,64,64]
    repeat 200x: h = W2 @ h + b2   (1x1 conv, shared weights)

Strategy:
  * The 200 repeated affine steps share one weight matrix, so the tail of
    the network is the affine map h -> W2^200 h + (sum_k W2^k) b2.  We fold
    W2^FUSE (computed in float64 on the host, rounded to f32) into the
    device program: the device runs 200/FUSE GEMM steps.  FUSE=200 folds
    everything into the conv weights (a single fused conv).  Numerics vs
    the f32 reference are ~1e-6 for every FUSE (spectral radius of W2 is
    0.979; all intermediate values stay tiny).
  * Data parallel across 8 NeuronCores: 16384 output positions -> 2048 per
    core (half an image each).  No cross-device communication.
  * Conv is an im2col GEMM with the bias folded in as a constant-1 row:
    K = 3*7*7 + 1 = 148, split into accumulating K=128 + K=20 matmuls.
  * Matmul dtype modes:
      - "f32r"  : TF32 operands (pre-rounded on host), 1 cycle/row.
      - "f32r3" : each operand split hi+lo TF32 terms; 3 matmuls
                  (hi*hi + hi*lo + lo*hi) -> fp32-grade accuracy at 3/8
                  the cost of true fp32 matmul.
      - "f32"   : plain fp32 (2 HW passes, 8 cycles/row).
    f32r matmuls require dst PSUM base partition 0; outputs land in four
    [64, 512] PSUM tiles and the Vector/Scalar engines copy them (with a
    +64 partition shift for the second position group) into the [128,
    1024] output layout.
"""

import numpy as np

import concourse.bacc as bacc
import concourse.tile as tile
import concourse.mybir as mybir
from concourse.bass_utils import run_bass_kernel_spmd

F32 = mybir.dt.float32
F32R = mybir.dt.float32r

N_CORES = 8
N_REPEAT = 200
FUSE = 200  # device runs N_REPEAT//FUSE GEMM steps; 200 -> fully folded conv
MM_DTYPE = "f32r3"  # "f32" | "f32r" | "f32r3"
WARMUP_MMS = 6  # dummy matmuls during the input DMA wait to warm the PE clock
POS_PER_CORE = 2048  # 4*64*64 / 8
HALF = POS_PER_CORE // 2  # free-dim size of the [128, 1024] output layout
OH = OW = 64
KH = KW = 7
CIN = 3
CH = 64
K_IM = CIN * KH * KW + 1  # 148: im2col rows + constant-1 bias row
K_LO = K_IM - 128  # 20

_cache = {}


def _build_nc(steps, mode):
    """Build + compile the per-core Bass program (same NEFF for all cores)."""
    nterm = {"f32": 1, "f32r": 1, "f32r3": 2}[mode]  # operand split terms
    mdt = F32 if mode == "f32" else F32R
    warmup = WARMUP_MMS if steps == 0 else 0
    nc = bacc.Bacc("TRN2", target_bir_lowering=False, debug=False,
                   num_devices=N_CORES)

    # conv weights: [K_IM, 64] lhsT layout, one tensor per split term
    wc_ext = [nc.declare_dram_parameter(f"wc{t}", [K_IM, CH], mdt, isOutput=False)
              for t in range(nterm)]
    im_ext = [nc.declare_dram_parameter(f"im{t}", [K_IM, POS_PER_CORE], mdt,
                                        isOutput=False)
              for t in range(nterm)]
    if steps:
        wl_ext = nc.declare_dram_parameter("wl", [128, CH + 1], F32, isOutput=False)
    o_ext = nc.declare_dram_parameter("o", [128, HALF], F32, isOutput=True)

    with tile.TileContext(nc) as tc:
        with (
            tc.tile_pool(name="const", bufs=1) as cpool,
            tc.tile_pool(name="act", bufs=2) as apool,
            tc.tile_pool(name="psum", bufs=1, space="PSUM") as ppool,
        ):
            wch = [cpool.tile([128, CH], mdt, name=f"wch{t}_sb") for t in range(nterm)]
            wcl = [cpool.tile([K_LO, CH], mdt, name=f"wcl{t}_sb") for t in range(nterm)]
            imh = [cpool.tile([128, POS_PER_CORE], mdt, name=f"imh{t}_sb")
                   for t in range(nterm)]
            iml = [cpool.tile([K_LO, POS_PER_CORE], mdt, name=f"iml{t}_sb")
                   for t in range(nterm)]
            for t in range(nterm):
                nc.sync.dma_start(wch[t][:], wc_ext[t][0:128, :])
                nc.sync.dma_start(wcl[t][:], wc_ext[t][128:K_IM, :])
            if steps:
                wl = cpool.tile([128, CH + 1], F32)
                nc.sync.dma_start(wl[:], wl_ext[:])
            # prime the scalar-engine activation table while DMAs run
            scratch = apool.tile([128, 1], F32, tag="scratch")
            nc.scalar.activation(scratch[:], wch[0][:, 0:1].bitcast(F32),
                                 mybir.ActivationFunctionType.Identity)
            if warmup:
                # warm up the PE clock (HAM) with dummy matmuls on the weights
                warm_ps = ppool.tile([64, 512], F32, name="warm_ps")
                warm_rhs = cpool.tile([128, 512], mybir.dt.bfloat16, name="warm_rhs")
                nc.vector.memset(warm_rhs[:], 0.0)
                for t in range(warmup):
                    nc.tensor.matmul(warm_ps[:], warm_rhs[:, 0:64], warm_rhs[:],
                                     start=True, stop=True, tile_position=(0, 0))
            tc.strict_bb_all_engine_barrier()
            # chunked im2col loads; conv chunk c depends only on its DMAs
            for t in range(nterm):
                for c in range(4):
                    cs = slice(c * 512, (c + 1) * 512)
                    nc.sync.dma_start(imh[t][:, cs], im_ext[t][0:128, cs])
                nc.sync.dma_start(iml[t][:], im_ext[t][128:K_IM, :])

            # ---- conv GEMM: 4 chunks of 512 positions, dst PSUM partitions 0:64
            if nterm == 1:
                pairs = [(0, 0)]
            else:  # hi*hi + hi*lo + lo*hi  (lo*lo term negligible)
                pairs = [(0, 0), (0, 1), (1, 0)]
            ps = [ppool.tile([64, 512], F32, name=f"ps{c}") for c in range(4)]
            for c in range(4):
                cs = slice(c * 512, (c + 1) * 512)
                n = len(pairs)
                for i, (tw, tx) in enumerate(pairs):
                    nc.tensor.matmul(ps[c][:], wch[tw][:], imh[tx][:, cs],
                                     start=(i == 0), stop=False,
                                     tile_position=(0, 0))
                    nc.tensor.matmul(ps[c][:], wcl[tw][:], iml[tx][:, cs],
                                     start=False, stop=(i == n - 1),
                                     tile_position=(0, 0))

            # ---- copies into [128, 1024] layout (+64 partition shift for B)
            h = apool.tile([128, HALF], F32, tag="h")
            nc.vector.tensor_copy(h[0:64, 0:512], ps[0][:])
            nc.scalar.copy(h[0:64, 512:1024], ps[1][:])
            nc.vector.tensor_copy(h[64:128, 0:512], ps[2][:])
            nc.scalar.copy(h[64:128, 512:1024], ps[3][:])
            if steps == 0:
                nc.sync.dma_start(o_ext[0:64, :], h[0:64, :])
                nc.sync.dma_start(o_ext[64:128, :], h[64:128, :])

            # ---- fused GEMM steps (fp32 exact): h <- P_FUSE @ h + c_FUSE
            for s in range(steps):
                bl = wl[:, CH:CH + 1]
                psA = ppool.tile([128, 512], F32, name="psA", bufs=2)
                psB = ppool.tile([128, 512], F32, name="psB", bufs=2)
                nc.tensor.matmul(psA[0:64, :], wl[0:64, 0:CH], h[0:64, 0:512],
                                 start=True, stop=True, tile_position=(0, 0))
                nc.tensor.matmul(psA[64:128, :], wl[0:64, 0:CH], h[0:64, 512:1024],
                                 start=True, stop=True, tile_position=(0, 64))
                nc.tensor.matmul(psB[0:64, :], wl[64:128, 0:CH], h[64:128, 0:512],
                                 start=True, stop=True, tile_position=(64, 0))
                nc.tensor.matmul(psB[64:128, :], wl[64:128, 0:CH],
                                 h[64:128, 512:1024],
                                 start=True, stop=True, tile_position=(64, 64))
                last = s == steps - 1
                h_new = apool.tile([128, HALF], F32, tag="h")
                nc.vector.tensor_scalar(h_new[:, 0:512], psA[:], bl, None,
                                        mybir.AluOpType.add)
                nc.scalar.activation(h_new[:, 512:1024], psB[:],
                                     mybir.ActivationFunctionType.Identity,
                                     bias=bl)
                if last:
                    nc.sync.dma_start(o_ext[:, 0:512], h_new[:, 0:512])
                    nc.sync.dma_start(o_ext[:, 512:1024], h_new[:, 512:1024])
                h = h_new

    nc.compile()
    return nc


def _fold(W1, b1, W2, b2, fuse):
    """Fold `fuse` affine steps into the conv weights (float64 host math).

    Returns (Wc [64,148] incl bias column, Pk [64,64]|None, ck [64]|None).
    """
    W2d = W2.astype(np.float64)
    W1m = W1.reshape(CH, K_IM - 1).astype(np.float64)

    def affine_pow(k):
        # (P, S) with P = W2^k, S = sum_{j<k} W2^j  via binary doubling
        P = np.eye(CH)
        S = np.zeros((CH, CH))
        base_P = W2d
        base_S = np.eye(CH)
        while k:
            if k & 1:
                S = base_S + base_P @ S
                P = base_P @ P
            base_S = base_S + base_P @ base_S
            base_P = base_P @ base_P
            k >>= 1
        return P, S

    if fuse == N_REPEAT:
        P, S = affine_pow(N_REPEAT)
        Wm = P @ W1m
        bias = P @ b1.astype(np.float64) + S @ b2.astype(np.float64)
    else:
        Wm = W1m
        bias = b1.astype(np.float64)
    Wc = np.concatenate([Wm, bias[:, None]], axis=1)  # [64, 148]
    if fuse == N_REPEAT:
        return Wc, None, None
    P, S = affine_pow(fuse)
    return Wc, P.astype(np.float32), (S @ b2.astype(np.float64)).astype(np.float32)


def _im2col_core(x, core):
    """im2col + constant-1 bias row for this core -> [148, 2048] f64->f32."""
    b = core // 2
    y0 = 32 * (core % 2)
    cols = np.empty((K_IM, POS_PER_CORE), np.float32)
    i = 0
    for c in range(CIN):
        for dy in range(KH):
            for dx in range(KW):
                cols[i] = x[b, c, y0 + dy:y0 + dy + 32, dx:dx + OW].reshape(-1)
                i += 1
    cols[i] = 1.0
    return cols


def _tf32_round(a):
    """Round f32 array to tf32 (10-bit mantissa), round-to-nearest-even."""
    a = np.ascontiguousarray(a, dtype=np.float32)
    u = a.view(np.uint32)
    lsb = (u >> 13) & 1
    out = ((u + 0x0FFF + lsb) & 0xFFFFE000).astype(np.uint32)
    return out.view(np.float32)


def _split_terms(a, mode):
    """Operand splitting per matmul dtype mode -> list of arrays."""
    if mode == "f32":
        return [np.ascontiguousarray(a, dtype=np.float32)]
    hi = _tf32_round(a)
    if mode == "f32r":
        return [hi]
    lo = _tf32_round(np.asarray(a, np.float32) - hi)
    return [hi, lo]


def _run(x, W1, b1, W2, b2, trace=False):
    x = np.asarray(x, dtype=np.float32)
    W1 = np.asarray(W1, dtype=np.float32)
    b1 = np.asarray(b1, dtype=np.float32)
    W2 = np.asarray(W2, dtype=np.float32)
    b2 = np.asarray(b2, dtype=np.float32)

    steps = 0 if FUSE == N_REPEAT else N_REPEAT // FUSE
    if steps:
        assert steps * FUSE == N_REPEAT

    key = (steps, MM_DTYPE, WARMUP_MMS)
    if _cache.get("key") != key:
        _cache["nc"] = _build_nc(steps, MM_DTYPE)
        _cache["key"] = key
    nc = _cache["nc"]

    nterm = {"f32": 1, "f32r": 1, "f32r3": 2}[MM_DTYPE]  # operand terms

    Wc, Pk, ck = _fold(W1, b1, W2, b2, FUSE)
    WcT = np.ascontiguousarray(Wc.T)  # [148, 64] lhsT layout
    w_terms = _split_terms(WcT, MM_DTYPE)

    shared = {f"wc{t}": w_terms[t] for t in range(len(w_terms))}
    if steps:
        PkT = np.ascontiguousarray(Pk.T)
        wl = np.concatenate([PkT, PkT], axis=0).astype(np.float32)
        bl = np.concatenate([ck, ck])[:, None].astype(np.float32)
        shared["wl"] = np.concatenate([wl, bl], axis=1)

    in_maps = []
    for core in range(N_CORES):
        cols = _im2col_core(x, core)
        x_terms = _split_terms(cols, MM_DTYPE)
        m = dict(shared)
        for t, arr in enumerate(x_terms):
            m[f"im{t}"] = arr
        in_maps.append(m)

    res = run_bass_kernel_spmd(nc, in_maps, list(range(N_CORES)), trace=trace)

    out = np.empty((4, CH, OH, OW), np.float32)
    for core in range(N_CORES):
        o = res.results[core]["o"].copy()
        if steps % 2 == 1:
            # undo the per-step quarter-block swap (Q2 <-> Q3)
            tmp = o[0:64, 512:1024].copy()
            o[0:64, 512:1024] = o[64:128, 0:512]
            o[64:128, 0:512] = tmp
        b = core // 2
        y0 = 32 * (core % 2)
        # group A = local positions 0..1023 (16 rows), group B = 1024..2047
        out[b, :, y0:y0 + 16, :] = o[0:64].reshape(CH, 16, OW)
        out[b, :, y0 + 16:y0 + 32, :] = o[64:128].reshape(CH, 16, OW)
    return out, res


def kernel(**inputs):
    out, _ = _run(inputs["x"], inputs["W1"], inputs["b1"],
                  inputs["W2"], inputs["b2"], trace=False)
    return out


def kernel_traced(**inputs):
    """Like kernel() but with NTFF hardware profiling; returns (out, res)."""
    import sys
    import types
    if "antenv.axon_hooks" not in sys.modules:
        from trn_agent_boot.trn_boot import _ntff_profile_via_ctypes
        hook = _ntff_profile_via_ctypes("/opt/axon/libaxon_pjrt.so")
        mod = types.ModuleType("antenv.axon_hooks")
        mod.get_axon_ntff_profile_hook = lambda: hook
        mod.set_axon_ntff_profile_hook = lambda h: None
        sys.modules["antenv.axon_hooks"] = mod
    return _run(inputs["x"], inputs["W1"], inputs["b1"],
                inputs["W2"], inputs["b2"], trace=True)


# revision 10
# speedup vs baseline: 1.5491x; 1.4203x over previous
"""Trainium2 Bass kernel for nn_Conv2D3_72026601554290.

Reference computation:
    h = conv7x7_valid(x[4,3,70,70], W1[64,3,7,7]) + b1      -> [4,64,64,64]
    repeat 200x: h = W2 @ h + b2   (1x1 conv, shared weights)

Strategy:
  * The 200 repeated affine steps share one weight matrix, so the tail of
    the network is the affine map h -> W2^200 h + (sum_k W2^k) b2.  We fold
    W2^FUSE (computed in float64 on the host, rounded to f32) into the
    device program: the device runs 200/FUSE GEMM steps.  FUSE=200 folds
    everything into the conv weights (a single fused conv).  Numerics vs
    the f32 reference are ~1e-6 for every FUSE (spectral radius of W2 is
    0.979; all intermediate values stay tiny).
  * Data parallel across 8 NeuronCores: 16384 output positions -> 2048 per
    core (half an image each).  No cross-device communication.
  * Conv is an im2col GEMM with the bias folded in as a constant-1 row:
    K = 3*7*7 + 1 = 148, split into accumulating K=128 + K=20 matmuls.
  * Matmul dtype modes:
      - "f32r"  : TF32 operands (pre-rounded on host), 1 cycle/row.
      - "f32r3" : each operand split hi+lo TF32 terms; 3 matmuls
                  (hi*hi + hi*lo + lo*hi) -> fp32-grade accuracy at 3/8
                  the cost of true fp32 matmul.
      - "f32"   : plain fp32 (2 HW passes, 8 cycles/row).
    f32r matmuls require dst PSUM base partition 0; outputs land in four
    [64, 512] PSUM tiles and the Vector/Scalar engines copy them (with a
    +64 partition shift for the second position group) into the [128,
    1024] output layout.
"""

import numpy as np

import concourse.bacc as bacc
import concourse.tile as tile
import concourse.mybir as mybir
from concourse.bass_utils import run_bass_kernel_spmd

F32 = mybir.dt.float32
F32R = mybir.dt.float32r

N_CORES = 8
N_REPEAT = 200
FUSE = 200  # device runs N_REPEAT//FUSE GEMM steps; 200 -> fully folded conv
MM_DTYPE = "f32r"  # "f32" | "f32r" | "f32r3"
WARMUP_MMS = 6  # dummy matmuls during the input DMA wait to warm the PE clock
POS_PER_CORE = 2048  # 4*64*64 / 8
HALF = POS_PER_CORE // 2  # free-dim size of the [128, 1024] output layout
OH = OW = 64
KH = KW = 7
CIN = 3
CH = 64
K_IM = CIN * KH * KW + 1  # 148: im2col rows + constant-1 bias row
K_LO = K_IM - 128  # 20

_cache = {}


def _build_nc(steps, mode):
    """Build + compile the per-core Bass program (same NEFF for all cores)."""
    nterm = {"f32": 1, "f32r": 1, "f32r3": 2}[mode]  # operand split terms
    mdt = F32 if mode == "f32" else F32R
    warmup = WARMUP_MMS if steps == 0 else 0
    nc = bacc.Bacc("TRN2", target_bir_lowering=False, debug=False,
                   num_devices=N_CORES)

    # conv weights: [K_IM, 64] lhsT layout, one tensor per split term
    wc_ext = [nc.declare_dram_parameter(f"wc{t}", [K_IM, CH], mdt, isOutput=False)
              for t in range(nterm)]
    im_ext = [nc.declare_dram_parameter(f"im{t}", [K_IM, POS_PER_CORE], mdt,
                                        isOutput=False)
              for t in range(nterm)]
    if steps:
        wl_ext = nc.declare_dram_parameter("wl", [128, CH + 1], F32, isOutput=False)
    o_ext = nc.declare_dram_parameter("o", [128, HALF], F32, isOutput=True)

    with tile.TileContext(nc) as tc:
        with (
            tc.tile_pool(name="const", bufs=1) as cpool,
            tc.tile_pool(name="act", bufs=2) as apool,
            tc.tile_pool(name="psum", bufs=1, space="PSUM") as ppool,
        ):
            wch = [cpool.tile([128, CH], mdt, name=f"wch{t}_sb") for t in range(nterm)]
            wcl = [cpool.tile([K_LO, CH], mdt, name=f"wcl{t}_sb") for t in range(nterm)]
            imh = [cpool.tile([128, POS_PER_CORE], mdt, name=f"imh{t}_sb")
                   for t in range(nterm)]
            iml = [cpool.tile([K_LO, POS_PER_CORE], mdt, name=f"iml{t}_sb")
                   for t in range(nterm)]
            for t in range(nterm):
                nc.sync.dma_start(wch[t][:], wc_ext[t][0:128, :])
                nc.sync.dma_start(wcl[t][:], wc_ext[t][128:K_IM, :])
            if steps:
                wl = cpool.tile([128, CH + 1], F32)
                nc.sync.dma_start(wl[:], wl_ext[:])
            # prime the scalar-engine activation table while DMAs run
            scratch = apool.tile([128, 1], F32, tag="scratch")
            nc.scalar.activation(scratch[:], wch[0][:, 0:1].bitcast(F32),
                                 mybir.ActivationFunctionType.Identity)
            if warmup:
                # warm up the PE clock (HAM) with dummy matmuls on the weights
                warm_ps = ppool.tile([64, 512], F32, name="warm_ps")
                warm_rhs = cpool.tile([128, 512], mybir.dt.bfloat16, name="warm_rhs")
                nc.vector.memset(warm_rhs[:], 0.0)
                for t in range(warmup):
                    nc.tensor.matmul(warm_ps[:], warm_rhs[:, 0:64], warm_rhs[:],
                                     start=True, stop=True, tile_position=(0, 0))
            tc.strict_bb_all_engine_barrier()
            # chunked im2col loads; conv chunk c depends only on its DMAs
            for t in range(nterm):
                for c in range(4):
                    cs = slice(c * 512, (c + 1) * 512)
                    nc.sync.dma_start(imh[t][:, cs], im_ext[t][0:128, cs])
                nc.sync.dma_start(iml[t][:], im_ext[t][128:K_IM, :])

            # ---- conv GEMM: 4 chunks of 512 positions, dst PSUM partitions 0:64
            if nterm == 1:
                pairs = [(0, 0)]
            else:  # hi*hi + hi*lo + lo*hi  (lo*lo term negligible)
                pairs = [(0, 0), (0, 1), (1, 0)]
            ps = [ppool.tile([64, 512], F32, name=f"ps{c}") for c in range(4)]
            for c in range(4):
                cs = slice(c * 512, (c + 1) * 512)
                n = len(pairs)
                for i, (tw, tx) in enumerate(pairs):
                    nc.tensor.matmul(ps[c][:], wch[tw][:], imh[tx][:, cs],
                                     start=(i == 0), stop=False,
                                     tile_position=(0, 0))
                    nc.tensor.matmul(ps[c][:], wcl[tw][:], iml[tx][:, cs],
                                     start=False, stop=(i == n - 1),
                                     tile_position=(0, 0))

            # ---- copies into [128, 1024] layout (+64 partition shift for B)
            h = apool.tile([128, HALF], F32, tag="h")
            nc.vector.tensor_copy(h[0:64, 0:512], ps[0][:])
            nc.scalar.copy(h[0:64, 512:1024], ps[1][:])
            nc.vector.tensor_copy(h[64:128, 0:512], ps[2][:])
            nc.scalar.copy(h[64:128, 512:1024], ps[3][:])
            if steps == 0:
                nc.sync.dma_start(o_ext[0:64, :], h[0:64, :])
                nc.sync.dma_start(o_ext[64:128, :], h[64:128, :])

            # ---- fused GEMM steps (fp32 exact): h <- P_FUSE @ h + c_FUSE
            for s in range(steps):
                bl = wl[:, CH:CH + 1]
                psA = ppool.tile([128, 512], F32, name="psA", bufs=2)
                psB = ppool.tile([128, 512], F32, name="psB", bufs=2)
                nc.tensor.matmul(psA[0:64, :], wl[0:64, 0:CH], h[0:64, 0:512],
                                 start=True, stop=True, tile_position=(0, 0))
                nc.tensor.matmul(psA[64:128, :], wl[0:64, 0:CH], h[0:64, 512:1024],
                                 start=True, stop=True, tile_position=(0, 64))
                nc.tensor.matmul(psB[0:64, :], wl[64:128, 0:CH], h[64:128, 0:512],
                                 start=True, stop=True, tile_position=(64, 0))
                nc.tensor.matmul(psB[64:128, :], wl[64:128, 0:CH],
                                 h[64:128, 512:1024],
                                 start=True, stop=True, tile_position=(64, 64))
                last = s == steps - 1
                h_new = apool.tile([128, HALF], F32, tag="h")
                nc.vector.tensor_scalar(h_new[:, 0:512], psA[:], bl, None,
                                        mybir.AluOpType.add)
                nc.scalar.activation(h_new[:, 512:1024], psB[:],
                                     mybir.ActivationFunctionType.Identity,
                                     bias=bl)
                if last:
                    nc.sync.dma_start(o_ext[:, 0:512], h_new[:, 0:512])
                    nc.sync.dma_start(o_ext[:, 512:1024], h_new[:, 512:1024])
                h = h_new

    nc.compile()
    return nc


def _fold(W1, b1, W2, b2, fuse):
    """Fold `fuse` affine steps into the conv weights (float64 host math).

    Returns (Wc [64,148] incl bias column, Pk [64,64]|None, ck [64]|None).
    """
    W2d = W2.astype(np.float64)
    W1m = W1.reshape(CH, K_IM - 1).astype(np.float64)

    def affine_pow(k):
        # (P, S) with P = W2^k, S = sum_{j<k} W2^j  via binary doubling
        P = np.eye(CH)
        S = np.zeros((CH, CH))
        base_P = W2d
        base_S = np.eye(CH)
        while k:
            if k & 1:
                S = base_S + base_P @ S
                P = base_P @ P
            base_S = base_S + base_P @ base_S
            base_P = base_P @ base_P
            k >>= 1
        return P, S

    if fuse == N_REPEAT:
        P, S = affine_pow(N_REPEAT)
        Wm = P @ W1m
        bias = P @ b1.astype(np.float64) + S @ b2.astype(np.float64)
    else:
        Wm = W1m
        bias = b1.astype(np.float64)
    Wc = np.concatenate([Wm, bias[:, None]], axis=1)  # [64, 148]
    if fuse == N_REPEAT:
        return Wc, None, None
    P, S = affine_pow(fuse)
    return Wc, P.astype(np.float32), (S @ b2.astype(np.float64)).astype(np.float32)


def _im2col_core(x, core):
    """im2col + constant-1 bias row for this core -> [148, 2048] f64->f32."""
    b = core // 2
    y0 = 32 * (core % 2)
    cols = np.empty((K_IM, POS_PER_CORE), np.float32)
    i = 0
    for c in range(CIN):
        for dy in range(KH):
            for dx in range(KW):
                cols[i] = x[b, c, y0 + dy:y0 + dy + 32, dx:dx + OW].reshape(-1)
                i += 1
    cols[i] = 1.0
    return cols


def _tf32_round(a):
    """Round f32 array to tf32 (10-bit mantissa), round-to-nearest-even."""
    a = np.ascontiguousarray(a, dtype=np.float32)
    u = a.view(np.uint32)
    lsb = (u >> 13) & 1
    out = ((u + 0x0FFF + lsb) & 0xFFFFE000).astype(np.uint32)
    return out.view(np.float32)


def _split_terms(a, mode):
    """Operand splitting per matmul dtype mode -> list of arrays."""
    if mode == "f32":
        return [np.ascontiguousarray(a, dtype=np.float32)]
    hi = _tf32_round(a)
    if mode == "f32r":
        return [hi]
    lo = _tf32_round(np.asarray(a, np.float32) - hi)
    return [hi, lo]


def _run(x, W1, b1, W2, b2, trace=False):
    x = np.asarray(x, dtype=np.float32)
    W1 = np.asarray(W1, dtype=np.float32)
    b1 = np.asarray(b1, dtype=np.float32)
    W2 = np.asarray(W2, dtype=np.float32)
    b2 = np.asarray(b2, dtype=np.float32)

    steps = 0 if FUSE == N_REPEAT else N_REPEAT // FUSE
    if steps:
        assert steps * FUSE == N_REPEAT

    key = (steps, MM_DTYPE, WARMUP_MMS)
    if _cache.get("key") != key:
        _cache["nc"] = _build_nc(steps, MM_DTYPE)
        _cache["key"] = key
    nc = _cache["nc"]

    nterm = {"f32": 1, "f32r": 1, "f32r3": 2}[MM_DTYPE]  # operand terms

    Wc, Pk, ck = _fold(W1, b1, W2, b2, FUSE)
    WcT = np.ascontiguousarray(Wc.T)  # [148, 64] lhsT layout
    w_terms = _split_terms(WcT, MM_DTYPE)

    shared = {f"wc{t}": w_terms[t] for t in range(len(w_terms))}
    if steps:
        PkT = np.ascontiguousarray(Pk.T)
        wl = np.concatenate([PkT, PkT], axis=0).astype(np.float32)
        bl = np.concatenate([ck, ck])[:, None].astype(np.float32)
        shared["wl"] = np.concatenate([wl, bl], axis=1)

    in_maps = []
    for core in range(N_CORES):
        cols = _im2col_core(x, core)
        x_terms = _split_terms(cols, MM_DTYPE)
        m = dict(shared)
        for t, arr in enumerate(x_terms):
            m[f"im{t}"] = arr
        in_maps.append(m)

    res = run_bass_kernel_spmd(nc, in_maps, list(range(N_CORES)), trace=trace)

    out = np.empty((4, CH, OH, OW), np.float32)
    for core in range(N_CORES):
        o = res.results[core]["o"].copy()
        if steps % 2 == 1:
            # undo the per-step quarter-block swap (Q2 <-> Q3)
            tmp = o[0:64, 512:1024].copy()
            o[0:64, 512:1024] = o[64:128, 0:512]
            o[64:128, 0:512] = tmp
        b = core // 2
        y0 = 32 * (core % 2)
        # group A = local positions 0..1023 (16 rows), group B = 1024..2047
        out[b, :, y0:y0 + 16, :] = o[0:64].reshape(CH, 16, OW)
        out[b, :, y0 + 16:y0 + 32, :] = o[64:128].reshape(CH, 16, OW)
    return out, res


def kernel(**inputs):
    out, _ = _run(inputs["x"], inputs["W1"], inputs["b1"],
                  inputs["W2"], inputs["b2"], trace=False)
    return out


def kernel_traced(**inputs):
    """Like kernel() but with NTFF hardware profiling; returns (out, res)."""
    import sys
    import types
    if "antenv.axon_hooks" not in sys.modules:
        from trn_agent_boot.trn_boot import _ntff_profile_via_ctypes
        hook = _ntff_profile_via_ctypes("/opt/axon/libaxon_pjrt.so")
        mod = types.ModuleType("antenv.axon_hooks")
        mod.get_axon_ntff_profile_hook = lambda: hook
        mod.set_axon_ntff_profile_hook = lambda h: None
        sys.modules["antenv.axon_hooks"] = mod
    return _run(inputs["x"], inputs["W1"], inputs["b1"],
                inputs["W2"], inputs["b2"], trace=True)


# revision 12
# speedup vs baseline: 1.6503x; 1.0653x over previous
"""Trainium2 Bass kernel for nn_Conv2D3_72026601554290.

Reference computation:
    h = conv7x7_valid(x[4,3,70,70], W1[64,3,7,7]) + b1      -> [4,64,64,64]
    repeat 200x: h = W2 @ h + b2   (1x1 conv, shared weights)

Strategy:
  * The 200 repeated affine steps share one weight matrix, so the tail of
    the network is the affine map h -> W2^200 h + (sum_k W2^k) b2.  We fold
    W2^FUSE (computed in float64 on the host, rounded to f32) into the
    device program: the device runs 200/FUSE GEMM steps.  FUSE=200 folds
    everything into the conv weights (a single fused conv).  Numerics vs
    the f32 reference are ~1e-6 for every FUSE (spectral radius of W2 is
    0.979; all intermediate values stay tiny).
  * Data parallel across 8 NeuronCores: 16384 output positions -> 2048 per
    core (half an image each).  No cross-device communication.
  * Conv is an im2col GEMM with the bias folded in as a constant-1 row:
    K = 3*7*7 + 1 = 148, split into accumulating K=128 + K=20 matmuls.
  * Matmul dtype modes:
      - "f32r"  : TF32 operands (pre-rounded on host), 1 cycle/row.
      - "f32r3" : each operand split hi+lo TF32 terms; 3 matmuls
                  (hi*hi + hi*lo + lo*hi) -> fp32-grade accuracy at 3/8
                  the cost of true fp32 matmul.
      - "f32"   : plain fp32 (2 HW passes, 8 cycles/row).
    f32r matmuls require dst PSUM base partition 0; outputs land in four
    [64, 512] PSUM tiles and the Vector/Scalar engines copy them (with a
    +64 partition shift for the second position group) into the [128,
    1024] output layout.
"""

import numpy as np

import concourse.bacc as bacc
import concourse.tile as tile
import concourse.mybir as mybir
from concourse.bass_utils import run_bass_kernel_spmd

F32 = mybir.dt.float32
F32R = mybir.dt.float32r

N_CORES = 8
N_REPEAT = 200
FUSE = 200  # device runs N_REPEAT//FUSE GEMM steps; 200 -> fully folded conv
MM_DTYPE = "f32r"  # "f32" | "f32r" | "f32r3"
WARMUP_MMS = 6  # dummy matmuls during the input DMA wait to warm the PE clock
POS_PER_CORE = 2048  # 4*64*64 / 8
HALF = POS_PER_CORE // 2  # free-dim size of the [128, 1024] output layout
OH = OW = 64
KH = KW = 7
CIN = 3
CH = 64
K_IM = CIN * KH * KW + 1  # 148: im2col rows + constant-1 bias row
K_LO = K_IM - 128  # 20

_cache = {}


def _build_nc(steps, mode):
    """Build + compile the per-core Bass program (same NEFF for all cores)."""
    nterm = {"f32": 1, "f32r": 1, "f32r3": 2}[mode]  # operand split terms
    mdt = F32 if mode == "f32" else F32R
    warmup = WARMUP_MMS if steps == 0 else 0
    nc = bacc.Bacc("TRN2", target_bir_lowering=False, debug=False,
                   num_devices=N_CORES)

    # conv weights: [K_IM, 64] lhsT layout, one tensor per split term
    wc_ext = [nc.declare_dram_parameter(f"wc{t}", [K_IM, CH], mdt, isOutput=False)
              for t in range(nterm)]
    im_ext = [nc.declare_dram_parameter(f"im{t}", [K_IM, POS_PER_CORE], mdt,
                                        isOutput=False)
              for t in range(nterm)]
    if steps:
        wl_ext = nc.declare_dram_parameter("wl", [128, CH + 1], F32, isOutput=False)
    o_ext = nc.declare_dram_parameter("o", [128, HALF], F32, isOutput=True)

    with tile.TileContext(nc) as tc:
        with (
            tc.tile_pool(name="const", bufs=1) as cpool,
            tc.tile_pool(name="act", bufs=2) as apool,
            tc.tile_pool(name="psum", bufs=1, space="PSUM") as ppool,
        ):
            wch = [cpool.tile([128, CH], mdt, name=f"wch{t}_sb") for t in range(nterm)]
            wcl = [cpool.tile([K_LO, CH], mdt, name=f"wcl{t}_sb") for t in range(nterm)]
            imh = [cpool.tile([128, POS_PER_CORE], mdt, name=f"imh{t}_sb")
                   for t in range(nterm)]
            iml = [cpool.tile([K_LO, POS_PER_CORE], mdt, name=f"iml{t}_sb")
                   for t in range(nterm)]
            for t in range(nterm):
                nc.sync.dma_start(wch[t][:], wc_ext[t][0:128, :])
                nc.sync.dma_start(wcl[t][:], wc_ext[t][128:K_IM, :])
            if steps:
                wl = cpool.tile([128, CH + 1], F32)
                nc.sync.dma_start(wl[:], wl_ext[:])
            # prime the scalar-engine activation table while DMAs run
            scratch = apool.tile([128, 1], F32, tag="scratch")
            nc.scalar.activation(scratch[:], wch[0][:, 0:1].bitcast(F32),
                                 mybir.ActivationFunctionType.Identity)
            if warmup:
                warm_rhs = cpool.tile([128, 512], mybir.dt.bfloat16, name="warm_rhs")
                nc.vector.memset(warm_rhs[:], 0.0)
            tc.strict_bb_all_engine_barrier()
            # chunked im2col loads, triggers split across the two HWDGE
            # engines (sync + scalar) so issue overhead parallelizes;
            # each conv chunk's matmuls depend only on its own DMAs
            for t in range(nterm):
                for c in range(4):
                    cs = slice(c * 512, (c + 1) * 512)
                    eng = nc.sync if c % 2 == 0 else nc.scalar
                    eng.dma_start(imh[t][:, cs], im_ext[t][0:128, cs])
                nc.sync.dma_start(iml[t][:], im_ext[t][128:K_IM, :])
            if warmup:
                # warm up the PE clock (HAM) while the im2col DMAs stream
                warm_ps = ppool.tile([64, 512], F32, name="warm_ps")
                for t in range(warmup):
                    nc.tensor.matmul(warm_ps[:], warm_rhs[:, 0:64], warm_rhs[:],
                                     start=True, stop=True, tile_position=(0, 0))

            # ---- conv GEMM: 4 chunks of 512 positions, dst PSUM partitions 0:64
            if nterm == 1:
                pairs = [(0, 0)]
            else:  # hi*hi + hi*lo + lo*hi  (lo*lo term negligible)
                pairs = [(0, 0), (0, 1), (1, 0)]
            ps = [ppool.tile([64, 512], F32, name=f"ps{c}") for c in range(4)]
            for c in range(4):
                cs = slice(c * 512, (c + 1) * 512)
                n = len(pairs)
                for i, (tw, tx) in enumerate(pairs):
                    nc.tensor.matmul(ps[c][:], wch[tw][:], imh[tx][:, cs],
                                     start=(i == 0), stop=False,
                                     tile_position=(0, 0))
                    nc.tensor.matmul(ps[c][:], wcl[tw][:], iml[tx][:, cs],
                                     start=False, stop=(i == n - 1),
                                     tile_position=(0, 0))

            # ---- copies into [128, 1024] layout (+64 partition shift for B),
            # each followed by its own output-store DMA
            h = apool.tile([128, HALF], F32, tag="h")
            nc.vector.tensor_copy(h[0:64, 0:512], ps[0][:])
            if steps == 0:
                nc.sync.dma_start(o_ext[0:64, 0:512], h[0:64, 0:512])
            nc.scalar.copy(h[0:64, 512:1024], ps[1][:])
            if steps == 0:
                nc.scalar.dma_start(o_ext[0:64, 512:1024], h[0:64, 512:1024])
            nc.vector.tensor_copy(h[64:128, 0:512], ps[2][:])
            if steps == 0:
                nc.sync.dma_start(o_ext[64:128, 0:512], h[64:128, 0:512])
            nc.scalar.copy(h[64:128, 512:1024], ps[3][:])
            if steps == 0:
                nc.scalar.dma_start(o_ext[64:128, 512:1024], h[64:128, 512:1024])

            # ---- fused GEMM steps (fp32 exact): h <- P_FUSE @ h + c_FUSE
            for s in range(steps):
                bl = wl[:, CH:CH + 1]
                psA = ppool.tile([128, 512], F32, name="psA", bufs=2)
                psB = ppool.tile([128, 512], F32, name="psB", bufs=2)
                nc.tensor.matmul(psA[0:64, :], wl[0:64, 0:CH], h[0:64, 0:512],
                                 start=True, stop=True, tile_position=(0, 0))
                nc.tensor.matmul(psA[64:128, :], wl[0:64, 0:CH], h[0:64, 512:1024],
                                 start=True, stop=True, tile_position=(0, 64))
                nc.tensor.matmul(psB[0:64, :], wl[64:128, 0:CH], h[64:128, 0:512],
                                 start=True, stop=True, tile_position=(64, 0))
                nc.tensor.matmul(psB[64:128, :], wl[64:128, 0:CH],
                                 h[64:128, 512:1024],
                                 start=True, stop=True, tile_position=(64, 64))
                last = s == steps - 1
                h_new = apool.tile([128, HALF], F32, tag="h")
                nc.vector.tensor_scalar(h_new[:, 0:512], psA[:], bl, None,
                                        mybir.AluOpType.add)
                nc.scalar.activation(h_new[:, 512:1024], psB[:],
                                     mybir.ActivationFunctionType.Identity,
                                     bias=bl)
                if last:
                    nc.sync.dma_start(o_ext[:, 0:512], h_new[:, 0:512])
                    nc.sync.dma_start(o_ext[:, 512:1024], h_new[:, 512:1024])
                h = h_new

    nc.compile()
    return nc


def _fold(W1, b1, W2, b2, fuse):
    """Fold `fuse` affine steps into the conv weights (float64 host math).

    Returns (Wc [64,148] incl bias column, Pk [64,64]|None, ck [64]|None).
    """
    W2d = W2.astype(np.float64)
    W1m = W1.reshape(CH, K_IM - 1).astype(np.float64)

    def affine_pow(k):
        # (P, S) with P = W2^k, S = sum_{j<k} W2^j  via binary doubling
        P = np.eye(CH)
        S = np.zeros((CH, CH))
        base_P = W2d
        base_S = np.eye(CH)
        while k:
            if k & 1:
                S = base_S + base_P @ S
                P = base_P @ P
            base_S = base_S + base_P @ base_S
            base_P = base_P @ base_P
            k >>= 1
        return P, S

    if fuse == N_REPEAT:
        P, S = affine_pow(N_REPEAT)
        Wm = P @ W1m
        bias = P @ b1.astype(np.float64) + S @ b2.astype(np.float64)
    else:
        Wm = W1m
        bias = b1.astype(np.float64)
    Wc = np.concatenate([Wm, bias[:, None]], axis=1)  # [64, 148]
    if fuse == N_REPEAT:
        return Wc, None, None
    P, S = affine_pow(fuse)
    return Wc, P.astype(np.float32), (S @ b2.astype(np.float64)).astype(np.float32)


def _im2col_core(x, core):
    """im2col + constant-1 bias row for this core -> [148, 2048] f64->f32."""
    b = core // 2
    y0 = 32 * (core % 2)
    cols = np.empty((K_IM, POS_PER_CORE), np.float32)
    i = 0
    for c in range(CIN):
        for dy in range(KH):
            for dx in range(KW):
                cols[i] = x[b, c, y0 + dy:y0 + dy + 32, dx:dx + OW].reshape(-1)
                i += 1
    cols[i] = 1.0
    return cols


def _tf32_round(a):
    """Round f32 array to tf32 (10-bit mantissa), round-to-nearest-even."""
    a = np.ascontiguousarray(a, dtype=np.float32)
    u = a.view(np.uint32)
    lsb = (u >> 13) & 1
    out = ((u + 0x0FFF + lsb) & 0xFFFFE000).astype(np.uint32)
    return out.view(np.float32)


def _split_terms(a, mode):
    """Operand splitting per matmul dtype mode -> list of arrays."""
    if mode == "f32":
        return [np.ascontiguousarray(a, dtype=np.float32)]
    hi = _tf32_round(a)
    if mode == "f32r":
        return [hi]
    lo = _tf32_round(np.asarray(a, np.float32) - hi)
    return [hi, lo]


def _run(x, W1, b1, W2, b2, trace=False):
    x = np.asarray(x, dtype=np.float32)
    W1 = np.asarray(W1, dtype=np.float32)
    b1 = np.asarray(b1, dtype=np.float32)
    W2 = np.asarray(W2, dtype=np.float32)
    b2 = np.asarray(b2, dtype=np.float32)

    steps = 0 if FUSE == N_REPEAT else N_REPEAT // FUSE
    if steps:
        assert steps * FUSE == N_REPEAT

    key = (steps, MM_DTYPE, WARMUP_MMS)
    if _cache.get("key") != key:
        _cache["nc"] = _build_nc(steps, MM_DTYPE)
        _cache["key"] = key
    nc = _cache["nc"]

    nterm = {"f32": 1, "f32r": 1, "f32r3": 2}[MM_DTYPE]  # operand terms

    Wc, Pk, ck = _fold(W1, b1, W2, b2, FUSE)
    WcT = np.ascontiguousarray(Wc.T)  # [148, 64] lhsT layout
    w_terms = _split_terms(WcT, MM_DTYPE)

    shared = {f"wc{t}": w_terms[t] for t in range(len(w_terms))}
    if steps:
        PkT = np.ascontiguousarray(Pk.T)
        wl = np.concatenate([PkT, PkT], axis=0).astype(np.float32)
        bl = np.concatenate([ck, ck])[:, None].astype(np.float32)
        shared["wl"] = np.concatenate([wl, bl], axis=1)

    in_maps = []
    for core in range(N_CORES):
        cols = _im2col_core(x, core)
        x_terms = _split_terms(cols, MM_DTYPE)
        m = dict(shared)
        for t, arr in enumerate(x_terms):
            m[f"im{t}"] = arr
        in_maps.append(m)

    res = run_bass_kernel_spmd(nc, in_maps, list(range(N_CORES)), trace=trace)

    out = np.empty((4, CH, OH, OW), np.float32)
    for core in range(N_CORES):
        o = res.results[core]["o"].copy()
        if steps % 2 == 1:
            # undo the per-step quarter-block swap (Q2 <-> Q3)
            tmp = o[0:64, 512:1024].copy()
            o[0:64, 512:1024] = o[64:128, 0:512]
            o[64:128, 0:512] = tmp
        b = core // 2
        y0 = 32 * (core % 2)
        # group A = local positions 0..1023 (16 rows), group B = 1024..2047
        out[b, :, y0:y0 + 16, :] = o[0:64].reshape(CH, 16, OW)
        out[b, :, y0 + 16:y0 + 32, :] = o[64:128].reshape(CH, 16, OW)
    return out, res


def kernel(**inputs):
    out, _ = _run(inputs["x"], inputs["W1"], inputs["b1"],
                  inputs["W2"], inputs["b2"], trace=False)
    return out


def kernel_traced(**inputs):
    """Like kernel() but with NTFF hardware profiling; returns (out, res)."""
    import sys
    import types
    if "antenv.axon_hooks" not in sys.modules:
        from trn_agent_boot.trn_boot import _ntff_profile_via_ctypes
        hook = _ntff_profile_via_ctypes("/opt/axon/libaxon_pjrt.so")
        mod = types.ModuleType("antenv.axon_hooks")
        mod.get_axon_ntff_profile_hook = lambda: hook
        mod.set_axon_ntff_profile_hook = lambda h: None
        sys.modules["antenv.axon_hooks"] = mod
    return _run(inputs["x"], inputs["W1"], inputs["b1"],
                inputs["W2"], inputs["b2"], trace=True)
